# revision 52
# baseline (speedup 1.0000x reference)
"""Trainium2 Bass kernel for nn_BBoxDecoder (HyperNetwork -> per-sample CoordinateNet).

Computation (reference):
    h1   = relu(z @ W1.T + b1)            (32, 512)
    h2   = relu(h1 @ W2.T + b2)           (32, 1024)
    flat = h2 @ W3.T + b3                 (32, 198916)   <- 815 MB of W3, the bottleneck
    per-sample 5-layer CoordinateNet on timestamps -> (32, 512, 4)

Distribution over 8 NeuronCores (fp16 single-plane; modeled 234472 ns vs
447868 ns fp16-pair baseline; HW rel err 1.053e-2 vs 2e-2 gate):
  - A single fp16 plane for W3 (pre-scaled x32) + fp16 h2 yields ~1e-2
    end-to-end, so W3 streams at 2 B/elem (~51 MB/core, the memory roofline)
    and the big matmul is ONE pass per (k-tile, chunk).
  - z and W1 are fp16 replicated (W1 x8 pre-scaled); W2 stays fp32; this
    keeps h2 ready ~17 us in so the W3 stream never stalls at startup.
  - W3 columns are sharded 8 ways and exchanged with 4 pipelined AllToAlls:
      group 0 ("small"): Win, b_in, b_h0..2, Wo, b_out  (4096 cols global)
      group 1+l: hidden-layer-l weight block Wh_l (65536 cols), peer-aligned so
      peer s carries input-rows i in [32s, 32s+32) -> extraction is one DMA.
  - CoordinateNet runs data-parallel (4 samples/core) fully in fp16 (weights
    straight from the A2A, fp16 activations, fp32 PSUM accumulate), 1 cyc/row.
  - flat-shard accumulators pack 4 sample-chunks per [128, 512] PSUM bank via
    matmul base-partition placement, so a whole 8192-col group stays resident.
  - Layer computations are interleaved into later groups' matmul streams at
    points where their AllToAll has already completed (the PE is in-order).

Host-side prep: every large input is pre-arranged into partition-major
[128, ...] contiguous layouts so device DMAs are clean large descriptors.
W3 columns are permuted so hidden-layer weights arrive input-major (i.e.
pre-transposed for the PE) and so that the PSUM->staging layout lands the
AllToAll buffer in plain row-major order.
"""

import os
import sys

import numpy as np

if os.path.isdir("/opt/trn_rl_repo") and "/opt/trn_rl_repo" not in sys.path:
    sys.path.insert(0, "/opt/trn_rl_repo")

import concourse.bass as bass
import concourse.mybir as mybir
import concourse.tile as tile
from concourse.bass import ts
from concourse.bass_utils import run_bass_kernel_spmd

# ---------------------------------------------------------------- constants
B = 32          # batch
NPTS = 512      # timestamps per sample
LAT = 4096      # latent dim
H1 = 512        # hyper hidden 1
H2 = 1024       # hyper hidden 2
HID = 256       # CoordinateNet hidden dim
P_TOTAL = 198916

NCORES = 8
BPC = B // NCORES         # 4 samples per core
W1PC = H1 // NCORES       # 64 W1 rows per core

# flat-param layout offsets (torch named_parameters order)
WH_OFF = [512 + l * (HID * HID + HID) for l in range(3)]  # 512, 66304, 132096
BH_OFF = [a + HID * HID for a in WH_OFF]                  # 66048, 131840, 197632
WO_OFF = 197888
BO_OFF = 198912

# stream groups (per-core column counts): small pieces, then 3 Wh blocks
SW = 512                  # small group cols per core (4096 global)
GW = HID * HID // NCORES  # 8192 Wh-block cols per core
GRP_W = [SW, GW, GW, GW]
GRP_OFF = [0, SW, SW + GW, SW + 2 * GW]
SCOLS = SW + 3 * GW       # 25088 per-core streamed W3 columns
NCH = [w // 512 for w in GRP_W]   # chunks of 512 per group: 1, 16, 16, 16

SCALE = 32.0              # h2 and W3 pre-scale; flat psum comes out x1024

FP = mybir.dt.float32
F16 = mybir.dt.float16
AF = mybir.ActivationFunctionType

# diagnostic ablations for TimelineSim probes (never set in production)
_ABLATE = frozenset(
    s for s in os.environ.get("K_ABLATE", "").split(",") if s
)

# scheduler virtual-time pins (ms) for the CoordinateNet layer placements:
# input, hidden0, hidden1, hidden2, output
_VT = [float(x) for x in os.environ.get(
    "K_VT", "0.120,0.155,0.200,0.205,0.210").split(",")]


# ------------------------------------------------------------- wait splitter
def _split_multi_waits(nc):
    """The walrus build here accepts at most one sync-wait per instruction.
    Engines execute in order, so hoisting all but the last wait onto fresh
    NOPs immediately before the instruction is semantically identical."""
    ctr = 0
    for f in nc.m.functions:
        for bb in f.blocks:
            out = []
            changed = False
            for ins in bb.instructions:
                si = getattr(ins, "sync_info", None)
                waits = list(si.on_wait) if (si is not None and si.on_wait) else []
                if len(waits) > 1:
                    changed = True
                    for w in waits[:-1]:
                        ctr += 1
                        out.append(
                            mybir.InstNoOp(
                                name=f"{ins.name}-sw{ctr}",
                                engine=ins.engine,
                                sync_info=mybir.SyncInfo(on_wait=[w], on_update=[]),
                            )
                        )
                    ins.sync_info = mybir.SyncInfo(
                        on_wait=waits[-1:], on_update=list(si.on_update or [])
                    )
                out.append(ins)
            if changed:
                try:
                    bb.instructions = out
                except Exception:
                    bb.instructions.clear()
                    bb.instructions.extend(out)


# ------------------------------------------------------------ device program
def _build_module(repeat: int = 1):
    nc = bass.Bass(num_devices=NCORES)

    zt_d = nc.dram_tensor("zt_r", [128, LAT // 128, B], F16, kind="ExternalInput")
    w1_d = nc.dram_tensor("w1r", [128, LAT // 128, H1], F16, kind="ExternalInput")
    b1_d = nc.dram_tensor("b1f", [128, H1 // 128], FP, kind="ExternalInput")
    w2_d = nc.dram_tensor("w2r", [128, H1 // 128, H2], FP, kind="ExternalInput")
    b2_d = nc.dram_tensor("b2s", [128, H2 // 128], FP, kind="ExternalInput")  # 32*b2
    w3_d = nc.dram_tensor("w3p", [128, 4, 2, SCOLS], F16, kind="ExternalInput")
    b3_d = nc.dram_tensor("b3r", [1, SCOLS], F16, kind="ExternalInput")  # 1024*b3
    ts_d = nc.dram_tensor("tst", [BPC, NPTS], F16, kind="ExternalInput")
    out_d = nc.dram_tensor("out", [BPC, NPTS, 4], FP, kind="ExternalOutput")

    with tile.TileContext(nc) as tc:
        with (
            tc.tile_pool(name="const", bufs=1) as const,
            tc.tile_pool(name="w3pool", bufs=6) as w3pool,
            tc.tile_pool(name="b3pool", bufs=1) as b3pool,
            tc.tile_pool(name="fsb", bufs=2) as fsb,
            tc.tile_pool(name="cpool", bufs=1) as cpool,
            tc.tile_pool(name="xpool", bufs=8) as xpool,
            tc.tile_pool(name="opool", bufs=4) as opool,
            tc.tile_pool(name="fpsum", bufs=5, space="PSUM") as fpsum,
            tc.tile_pool(name="mpsum", bufs=3, space="PSUM") as mpsum,
            tc.tile_pool(name="dram", bufs=1, space="DRAM") as dram,
        ):
            for _rep in range(repeat):
                _emit_body(nc, tc, const, w3pool, b3pool, fsb, cpool, xpool,
                           opool, fpsum, mpsum, dram,
                           zt_d, w1_d, b1_d, w2_d, b2_d, w3_d, b3_d, ts_d, out_d)

    _split_multi_waits(nc)
    return nc


def _emit_body(nc, tc, const, w3pool, b3pool, fsb, cpool, xpool, opool,
               fpsum, mpsum, dram, zt_d, w1_d, b1_d, w2_d, b2_d, w3_d, b3_d,
               ts_d, out_d):
    # ---- constant loads (all host-prearranged partition-major).
    # z/W1 (both fp16, W1 pre-scaled x8) are chunked so the h1 matmuls start
    # as soon as the first quarter lands.
    zchunks, w1chunks = [], []
    for c4 in range(4):
        zc = const.tile([128, 8, B], F16, name=f"zsb{c4}", tag=f"zsb{c4}")
        nc.sync.dma_start(zc[:], zt_d[:, ts(c4, 8), :])
        w1c_t = const.tile([128, 8, H1], F16, name=f"w1sb{c4}", tag=f"w1sb{c4}")
        nc.sync.dma_start(w1c_t[:], w1_d[:, ts(c4, 8), :])
        zchunks.append(zc)
        w1chunks.append(w1c_t)
    w2sb = const.tile([128, H1 // 128, H2], FP, name="w2sb", tag="w2sb")
    nc.sync.dma_start(w2sb[:], w2_d[:, :, :])
    b1sb = const.tile([128, H1 // 128], FP, name="b1sb", tag="b1sb")
    nc.sync.dma_start(b1sb[:], b1_d[:, :])
    b2sb = const.tile([128, H2 // 128], FP, name="b2sb", tag="b2sb")
    nc.sync.dma_start(b2sb[:], b2_d[:, :])
    tssb = const.tile([1, BPC, NPTS], F16, name="tssb", tag="tssb")
    nc.sync.dma_start(tssb[:], ts_d[:, :].rearrange("(a j) n -> a j n", a=1))
    ones16 = const.tile([1, 128], F16, name="ones16", tag="ones16")
    nc.gpsimd.memset(ones16[:], 1.0)

    # ---- h1 = relu(W1 @ z.T + b1), stored [128, 4, 32] fp32.
    # W1/z are fp16 (W1 x8 pre-scaled); the ACT rescales by 1/8 exactly.
    h1sb = const.tile([128, H1 // 128, B], FP, name="h1sb", tag="h1sb")
    for m in range(H1 // 128):
        h1ps = mpsum.tile([128, B], FP, name="h1ps", tag="ps")
        for kt in range(LAT // 128):
            nc.tensor.matmul(
                h1ps[:],
                w1chunks[kt // 8][:, kt % 8, ts(m, 128)],
                zchunks[kt // 8][:, kt % 8, :],
                start=(kt == 0), stop=(kt == LAT // 128 - 1),
            )
        nc.scalar.activation(
            h1sb[:, m, :], h1ps[:], AF.Relu, bias=b1sb[:, m : m + 1],
            scale=1.0 / 8.0,
        )

    # ---- h2h = fp16(32*relu(W2 @ h1 + b2)) as [128, 8, 32]
    h2h = const.tile([128, H2 // 128, B], F16, name="h2h", tag="h2h")
    for m in range(H2 // 128):
        h2ps = mpsum.tile([128, B], FP, name="h2ps", tag="ps")
        for kt in range(H1 // 128):
            nc.tensor.matmul(
                h2ps[:], w2sb[:, kt, ts(m, 128)], h1sb[:, kt, :],
                start=(kt == 0), stop=(kt == H1 // 128 - 1),
            )
        # 32*relu(x + b2) == relu(32x + 32*b2); b2s is pre-scaled on host
        nc.scalar.activation(
            h2h[:, m, :], h2ps[:], AF.Relu, bias=b2sb[:, m : m + 1], scale=SCALE
        )

    # ---- CoordinateNet param tiles (fp16, filled by extraction DMAs)
    win4 = cpool.tile([1, BPC, HID], F16, name="win4", tag="win4")
    bin4 = cpool.tile([128, BPC, 2], F16, name="bin4", tag="bin4")
    wh4s = [
        cpool.tile([128, BPC, 2, HID], F16, name=f"wh4_{l}", tag=f"wh4_{l}")
        for l in range(3)
    ]
    bh4s = [
        cpool.tile([128, BPC, 2], F16, name=f"bh4_{l}", tag=f"bh4_{l}")
        for l in range(3)
    ]
    wo4 = cpool.tile([128, BPC, 2, 4], F16, name="wo4", tag="wo4")
    bo4 = cpool.tile([1, BPC, 4], F16, name="bo4", tag="bo4")

    xs = [None] * BPC

    def _input_layer():
        for j in range(BPC):
            xc = xpool.tile([128, 2, NPTS], F16, name="xt", tag="xt")
            for t in range(2):
                xps = mpsum.tile([128, NPTS], FP, name="xps", tag="ps")
                nc.tensor.matmul(
                    xps[:], win4[0:1, j, ts(t, 128)], tssb[0:1, j, :],
                    start=True, stop=True,
                )
                nc.scalar.activation(
                    xc[:, t, :], xps[:], AF.Relu, bias=bin4[:, j, t : t + 1]
                )
            xs[j] = xc

    def _hidden_layer(l):
        for j in range(BPC):
            xn = xpool.tile([128, 2, NPTS], F16, name="xt", tag="xt")
            for m in range(2):
                hps = mpsum.tile([128, NPTS], FP, name="hps", tag="ps")
                for t in range(2):
                    nc.tensor.matmul(
                        hps[:], wh4s[l][:, j, t, ts(m, 128)], xs[j][:, t, :],
                        start=(t == 0), stop=(t == 1),
                    )
                nc.scalar.activation(
                    xn[:, m, :], hps[:], AF.Relu, bias=bh4s[l][:, j, m : m + 1]
                )
            xs[j] = xn

    def _output_layer():
        # one [128, 16] psum per 128-point chunk m holds all 4 samples' 4 outs
        for m in range(4):
            ops_ = mpsum.tile([128, 4 * BPC], FP, name="ops", tag="ps")
            for j in range(BPC):
                for t in range(2):
                    nc.tensor.matmul(
                        ops_[:, ts(j, 4)], xs[j][:, t, ts(m, 128)],
                        wo4[:, j, t, :], start=(t == 0), stop=False,
                    )
                nc.tensor.matmul(
                    ops_[:, ts(j, 4)], ones16[:, :128], bo4[0:1, j, :],
                    start=False, stop=True,
                )
            outm = opool.tile([128, BPC, 4], FP, name="outm", tag="outm")
            nc.scalar.activation(
                outm[:], ops_[:].rearrange("p (j o) -> p j o", o=4), AF.Sigmoid
            )
            nc.sync.dma_start(
                out_d[:, ts(m, 128), :].rearrange("j p o -> p j o"), outm[:]
            )

    def _extract_small(a2a_out):
        # peer layout (512 cols each): 0: Win|bin, 1: bh0|bh1, 2: bh2|bo|pad,
        # 3: Wo[i<128], 4: Wo[i>=128], 5-7: pad
        v = nc.gpsimd
        v.dma_start(
            win4[:], a2a_out[0:BPC, 0:HID].rearrange("(a r) o -> a r o", a=1)
        )
        for dst, r0, col0 in (
            (bin4, 0, HID),
            (bh4s[0], BPC, 0),
            (bh4s[1], BPC, HID),
            (bh4s[2], 2 * BPC, 0),
        ):
            for t in range(2):
                v.dma_start(
                    dst[:, :, t : t + 1],
                    a2a_out[
                        r0 : r0 + BPC, col0 + 128 * t : col0 + 128 * (t + 1)
                    ].rearrange("r p -> p r"),
                )
        v.dma_start(
            bo4[:],
            a2a_out[2 * BPC : 3 * BPC, HID : HID + 4].rearrange(
                "(a r) o -> a r o", a=1
            ),
        )
        for t in range(2):
            pr = (3 + t) * BPC
            v.dma_start(
                wo4[:, :, t, :],
                a2a_out[pr : pr + BPC, :].rearrange("r (i o) -> i r o", o=4),
            )

    def _extract_wh(l, a2a_out):
        # peer s carries input-rows i in [32s, 32s+32), each row 256 cols;
        # i = 128*t + 32*s4 + il -> partition 32*s4+il, half t.
        # Queue choice: each extraction fires right after its AllToAll, so it
        # must sit on a queue idle at that moment. wh0 fires mid-stream ->
        # Act queue (its flushes are far away); wh1/wh2 fire after the W3
        # stream is done -> SP + Act split (Pool stays free for the next
        # group's staging + collective).
        for t in range(2):
            if l < 2:
                eng = nc.gpsimd
            else:
                eng = nc.sync if t == 0 else nc.scalar
            for s4 in range(4):
                pr = (4 * t + s4) * BPC
                eng.dma_start(
                    wh4s[l][32 * s4 : 32 * s4 + 32, :, t, :],
                    a2a_out[pr : pr + BPC, :].rearrange(
                        "r (il o) -> il r o", o=HID
                    ),
                )

    # ---- streamed W3 shard matmul + pipelined AllToAll param exchange.
    # PE emission order interleaves CoordinateNet layers where their params
    # are already exchanged, so the in-order PE never stalls.
    a2a_outs = []
    for g in range(4):
        gw = GRP_W[g]
        c0 = GRP_OFF[g]
        nch = NCH[g]
        ntile = (nch + 3) // 4
        fpt = [
            fpsum.tile([128, 512], FP, name=f"fpt{g}_{i}", tag="fps")
            for i in range(ntile)
        ]
        nq = max(1, gw // 2048)       # quarter-stripes: fine-grained DMA FIFO
        qw = gw // nq
        for kk in range(4):
            w3q = []
            for q4 in range(nq):
                w3t = w3pool.tile([128, 2, qw], F16, name="w3sb", tag="w3sb")
                nc.sync.dma_start(
                    w3t[:],
                    w3_d[:, kk, :, c0 + q4 * qw : c0 + (q4 + 1) * qw],
                )
                w3q.append(w3t)
            for t in range(2):
                k = kk * 2 + t
                for j in range(nch):
                    po = 32 * (j % 4)
                    nc.tensor.matmul(
                        fpt[j // 4][po : po + 32, :],
                        h2h[:, k, :],
                        w3q[j // 4][:, t, ts(j % 4, 512)] if nch >= 4
                        else w3q[0][:, t, ts(j, 512)],
                        start=(k == 0),
                        stop=False,
                        tile_position=(0, po),
                    )
            # stage net layers where the PE arrives well after the layer's
            # AllToAll + extraction have completed. tile_wait_until pins the
            # scheduler's placement (its internal collective-cost model is
            # optimistic and would otherwise hoist these into the stream,
            # stalling the in-order PE on the exchange).
            if "nocoord" not in _ABLATE:
                if kk == 1 and g == 2:
                    with tc.tile_wait_until(_VT[0]):
                        _input_layer()
                if kk == 1 and g == 3:
                    with tc.tile_wait_until(_VT[1]):
                        _hidden_layer(0)

        b3sb = b3pool.tile([1, gw], F16, name="b3sb", tag="b3sb")
        nc.sync.dma_start(b3sb[:], b3_d[:, c0 : c0 + gw])
        flat_sb = fsb.tile([128, 512 * ntile], F16, name="flat_sb", tag="flat_sb")
        for j in range(nch):
            po = 32 * (j % 4)
            nc.tensor.matmul(
                fpt[j // 4][po : po + 32, :],
                ones16[:, :B],
                b3sb[:, ts(j, 512)],
                start=False,
                stop=True,
                tile_position=(0, po),
            )
        for t4 in range(ntile):
            # undo the 32*32 pre-scale (exact power of two), cast to fp16;
            # one ACT flushes a whole PSUM tile (all 4 sample-slots)
            nc.scalar.mul(
                flat_sb[:, ts(t4, 512)] if nch >= 4 else flat_sb[0:32, 0:512],
                fpt[t4][:] if nch >= 4 else fpt[t4][0:32, :],
                1.0 / 1024.0,
            )
        a2a_in = dram.tile([B, gw], F16, name=f"a2ain{g}", tag=f"a2ain{g}")
        a2a_out = dram.tile([B, gw], F16, name=f"a2aout{g}", tag=f"a2aout{g}")
        fw = 512 * ntile
        for f in range(min(4, nch)):
            # the last group's staging is latency-critical (it gates the
            # final AllToAll after the stream is over): use the then-idle
            # SP/Act HWDGE queues there instead of serial Pool SWDGE gens
            seng = (nc.sync if f % 2 == 0 else nc.scalar) if g == 3 else nc.gpsimd
            seng.dma_start(
                a2a_in[:, f * fw : (f + 1) * fw] if nch >= 4
                else a2a_in[:, 0:gw],
                flat_sb[32 * f : 32 * f + 32, 0:fw] if nch >= 4
                else flat_sb[0:32, 0:gw],
            )
        if "nocoll" in _ABLATE:
            a2a_out = a2a_in
        else:
            nc.gpsimd.collective_compute(
                "AllToAll",
                mybir.AluOpType.bypass,
                replica_groups=[list(range(NCORES))],
                ins=[a2a_in.opt()],
                outs=[a2a_out.opt()],
            )
        a2a_outs.append(a2a_out)
        if "nocoord" not in _ABLATE:
            if g == 0:
                _extract_small(a2a_out)
            elif g == 1:
                _extract_wh(0, a2a_out)
            elif g == 3:
                # wh1's extraction is emitted here -- after group 3's staging
                # and AllToAll are queued -- so its descriptor generation
                # cannot delay them on the Pool queue; hidden layer 1 then
                # runs on the PE while the last AllToAll is in flight.
                _extract_wh(1, a2a_outs[2])
                with tc.tile_wait_until(_VT[2]):
                    _hidden_layer(1)
                _extract_wh(2, a2a_out)

    if "nocoord" not in _ABLATE:
        with tc.tile_wait_until(_VT[3]):
            _hidden_layer(2)
        with tc.tile_wait_until(_VT[4]):
            _output_layer()


_NC_CACHE = {}


def _get_module(repeat: int = 1):
    if repeat not in _NC_CACHE:
        _NC_CACHE[repeat] = _build_module(repeat)
    return _NC_CACHE[repeat]


# -------------------------------------------------------------- host wrapper
def _core_gcols(c):
    """Global W3-column index (or -1 for zero pad) of every streamed column,
    in per-core storage order."""
    small = np.full((NCORES, SW), -1, np.int64)
    small[0] = np.arange(512)                       # Win (o-major) | bin
    small[1, 0:HID] = BH_OFF[0] + np.arange(HID)    # bh0
    small[1, HID:2 * HID] = BH_OFF[1] + np.arange(HID)  # bh1
    small[2, 0:HID] = BH_OFF[2] + np.arange(HID)    # bh2
    small[2, HID : HID + 4] = BO_OFF + np.arange(4)  # bo
    for pk in (3, 4):
        ii = (pk - 3) * 128 + np.arange(128)
        # Wo i-major: col (i, o) <- WO_OFF + o*HID + i
        small[pk] = (WO_OFF + np.arange(4)[None, :] * HID + ii[:, None]).ravel()

    # Wh blocks: storage chunk j holds param chunk k = (j%4)*4 + j//4 so the
    # PSUM partition-slot staging lands the A2A buffer in plain param order.
    j = np.arange(16)
    k = (j % 4) * 4 + j // 4
    s_of_store = (k[:, None] * 512 + np.arange(512)[None, :]).ravel()
    il, o = s_of_store // HID, s_of_store % HID
    parts = [small[c]]
    for l in range(3):
        parts.append(WH_OFF[l] + o * HID + (32 * c + il))
    return np.concatenate(parts)


_PREP_CACHE = None
LAST_RESULTS = None


def prepare_in_maps(z, timestamps, W1, b1, W2, b2, W3, b3):
    z = np.asarray(z, np.float32)
    timestamps = np.asarray(timestamps, np.float32)
    W1 = np.asarray(W1, np.float32)
    b1 = np.asarray(b1, np.float32)
    W2 = np.asarray(W2, np.float32)
    b2 = np.asarray(b2, np.float32)
    W3 = np.asarray(W3, np.float32)
    b3 = np.asarray(b3, np.float32)

    zt_r = np.ascontiguousarray(
        z.T.reshape(LAT // 128, 128, B).transpose(1, 0, 2)
    ).astype(np.float16)
    w1r = np.ascontiguousarray(
        (8.0 * W1).T.reshape(LAT // 128, 128, H1).transpose(1, 0, 2)
    ).astype(np.float16)
    b1f = np.ascontiguousarray(b1.reshape(H1 // 128, 128).T)
    w2r = np.ascontiguousarray(
        W2.T.reshape(H1 // 128, 128, H2).transpose(1, 0, 2)
    )
    b2s = np.ascontiguousarray((SCALE * b2).reshape(H2 // 128, 128).T)

    in_maps = []
    for c in range(NCORES):
        gcols = _core_gcols(c)
        valid = gcols >= 0
        gc = np.where(valid, gcols, 0)
        wsel = SCALE * W3[gc, :]
        wsel[~valid] = 0.0
        w3p = np.ascontiguousarray(
            wsel.T.reshape(4, 2, 128, SCOLS).transpose(2, 0, 1, 3)
        ).astype(np.float16)
        b3sel = 1024.0 * b3[gc]
        b3sel[~valid] = 0.0
        b3r = b3sel.astype(np.float16).reshape(1, SCOLS)
        in_maps.append(
            {
                "zt_r": zt_r,
                "w1r": w1r,
                "b1f": b1f,
                "w2r": w2r,
                "b2s": b2s,
                "w3p": w3p,
                "b3r": b3r,
                "tst": np.ascontiguousarray(
                    timestamps[c * BPC : (c + 1) * BPC, :, 0]
                ).astype(np.float16),
            }
        )
    return in_maps


def kernel(z, timestamps, W1, b1, W2, b2, W3, b3):
    global LAST_RESULTS
    in_maps = prepare_in_maps(z, timestamps, W1, b1, W2, b2, W3, b3)
    nc = _get_module(1)
    res = run_bass_kernel_spmd(nc, in_maps, core_ids=list(range(NCORES)))
    LAST_RESULTS = res
    out = np.concatenate(
        [np.asarray(res.results[c]["out"]) for c in range(NCORES)], axis=0
    )
    return out.astype(np.float32, copy=False)


# revision 53
# speedup vs baseline: 1.0084x; 1.0084x over previous
"""Trainium2 Bass kernel for nn_BBoxDecoder (HyperNetwork -> per-sample CoordinateNet).

Computation (reference):
    h1   = relu(z @ W1.T + b1)            (32, 512)
    h2   = relu(h1 @ W2.T + b2)           (32, 1024)
    flat = h2 @ W3.T + b3                 (32, 198916)   <- 815 MB of W3, the bottleneck
    per-sample 5-layer CoordinateNet on timestamps -> (32, 512, 4)

Distribution over 8 NeuronCores (fp16 single-plane; modeled 234472 ns vs
447868 ns fp16-pair baseline; HW rel err 1.053e-2 vs 2e-2 gate):
  - A single fp16 plane for W3 (pre-scaled x32) + fp16 h2 yields ~1e-2
    end-to-end, so W3 streams at 2 B/elem (~51 MB/core, the memory roofline)
    and the big matmul is ONE pass per (k-tile, chunk).
  - z and W1 are fp16 replicated (W1 x8 pre-scaled); W2 stays fp32; this
    keeps h2 ready ~17 us in so the W3 stream never stalls at startup.
  - W3 columns are sharded 8 ways and exchanged with 4 pipelined AllToAlls:
      group 0 ("small"): Win, b_in, b_h0..2, Wo, b_out  (4096 cols global)
      group 1+l: hidden-layer-l weight block Wh_l (65536 cols), peer-aligned so
      peer s carries input-rows i in [32s, 32s+32) -> extraction is one DMA.
  - CoordinateNet runs data-parallel (4 samples/core) fully in fp16 (weights
    straight from the A2A, fp16 activations, fp32 PSUM accumulate), 1 cyc/row.
  - flat-shard accumulators pack 4 sample-chunks per [128, 512] PSUM bank via
    matmul base-partition placement, so a whole 8192-col group stays resident.
  - Layer computations are interleaved into later groups' matmul streams at
    points where their AllToAll has already completed (the PE is in-order).

Host-side prep: every large input is pre-arranged into partition-major
[128, ...] contiguous layouts so device DMAs are clean large descriptors.
W3 columns are permuted so hidden-layer weights arrive input-major (i.e.
pre-transposed for the PE) and so that the PSUM->staging layout lands the
AllToAll buffer in plain row-major order.
"""

import os
import sys

import numpy as np

if os.path.isdir("/opt/trn_rl_repo") and "/opt/trn_rl_repo" not in sys.path:
    sys.path.insert(0, "/opt/trn_rl_repo")

import concourse.bass as bass
import concourse.mybir as mybir
import concourse.tile as tile
from concourse.bass import ts
from concourse.bass_utils import run_bass_kernel_spmd

# ---------------------------------------------------------------- constants
B = 32          # batch
NPTS = 512      # timestamps per sample
LAT = 4096      # latent dim
H1 = 512        # hyper hidden 1
H2 = 1024       # hyper hidden 2
HID = 256       # CoordinateNet hidden dim
P_TOTAL = 198916

NCORES = 8
BPC = B // NCORES         # 4 samples per core
W1PC = H1 // NCORES       # 64 W1 rows per core

# flat-param layout offsets (torch named_parameters order)
WH_OFF = [512 + l * (HID * HID + HID) for l in range(3)]  # 512, 66304, 132096
BH_OFF = [a + HID * HID for a in WH_OFF]                  # 66048, 131840, 197632
WO_OFF = 197888
BO_OFF = 198912

# stream groups (per-core column counts): small pieces, then 3 Wh blocks
SW = 512                  # small group cols per core (4096 global)
GW = HID * HID // NCORES  # 8192 Wh-block cols per core
GRP_W = [SW, GW, GW, GW]
GRP_OFF = [0, SW, SW + GW, SW + 2 * GW]
SCOLS = SW + 3 * GW       # 25088 per-core streamed W3 columns
NCH = [w // 512 for w in GRP_W]   # chunks of 512 per group: 1, 16, 16, 16

SCALE = 32.0              # h2 and W3 pre-scale; flat psum comes out x1024

FP = mybir.dt.float32
F16 = mybir.dt.float16
AF = mybir.ActivationFunctionType

# diagnostic ablations for TimelineSim probes (never set in production)
_ABLATE = frozenset(
    s for s in os.environ.get("K_ABLATE", "").split(",") if s
)

# scheduler virtual-time pins (ms) for the CoordinateNet layer placements:
# input, hidden0, hidden1, hidden2, output
_VT = [float(x) for x in os.environ.get(
    "K_VT", "0.120,0.155,0.200,0.205,0.210").split(",")]


# ------------------------------------------------------------- wait splitter
def _split_multi_waits(nc):
    """The walrus build here accepts at most one sync-wait per instruction.
    Engines execute in order, so hoisting all but the last wait onto fresh
    NOPs immediately before the instruction is semantically identical."""
    ctr = 0
    for f in nc.m.functions:
        for bb in f.blocks:
            out = []
            changed = False
            for ins in bb.instructions:
                si = getattr(ins, "sync_info", None)
                waits = list(si.on_wait) if (si is not None and si.on_wait) else []
                if len(waits) > 1:
                    changed = True
                    for w in waits[:-1]:
                        ctr += 1
                        out.append(
                            mybir.InstNoOp(
                                name=f"{ins.name}-sw{ctr}",
                                engine=ins.engine,
                                sync_info=mybir.SyncInfo(on_wait=[w], on_update=[]),
                            )
                        )
                    ins.sync_info = mybir.SyncInfo(
                        on_wait=waits[-1:], on_update=list(si.on_update or [])
                    )
                out.append(ins)
            if changed:
                try:
                    bb.instructions = out
                except Exception:
                    bb.instructions.clear()
                    bb.instructions.extend(out)


# ------------------------------------------------------------ device program
def _build_module(repeat: int = 1):
    nc = bass.Bass(num_devices=NCORES)

    zt_d = nc.dram_tensor("zt_r", [128, LAT // 128, B], F16, kind="ExternalInput")
    w1_d = nc.dram_tensor("w1r", [128, LAT // 128, H1], F16, kind="ExternalInput")
    b1_d = nc.dram_tensor("b1f", [128, H1 // 128], FP, kind="ExternalInput")
    w2_d = nc.dram_tensor("w2r", [128, H1 // 128, H2], FP, kind="ExternalInput")
    b2_d = nc.dram_tensor("b2s", [128, H2 // 128], FP, kind="ExternalInput")  # 32*b2
    w3_d = nc.dram_tensor("w3p", [128, 4, 2, SCOLS], F16, kind="ExternalInput")
    b3_d = nc.dram_tensor("b3r", [1, SCOLS], F16, kind="ExternalInput")  # 1024*b3
    ts_d = nc.dram_tensor("tst", [BPC, NPTS], F16, kind="ExternalInput")
    out_d = nc.dram_tensor("out", [BPC, NPTS, 4], FP, kind="ExternalOutput")

    with tile.TileContext(nc) as tc:
        with (
            tc.tile_pool(name="const", bufs=1) as const,
            tc.tile_pool(name="w3pool", bufs=6) as w3pool,
            tc.tile_pool(name="b3pool", bufs=1) as b3pool,
            tc.tile_pool(name="fsb", bufs=2) as fsb,
            tc.tile_pool(name="cpool", bufs=1) as cpool,
            tc.tile_pool(name="xpool", bufs=8) as xpool,
            tc.tile_pool(name="opool", bufs=4) as opool,
            tc.tile_pool(name="fpsum", bufs=5, space="PSUM") as fpsum,
            tc.tile_pool(name="mpsum", bufs=3, space="PSUM") as mpsum,
            tc.tile_pool(name="dram", bufs=1, space="DRAM") as dram,
        ):
            for _rep in range(repeat):
                _emit_body(nc, tc, const, w3pool, b3pool, fsb, cpool, xpool,
                           opool, fpsum, mpsum, dram,
                           zt_d, w1_d, b1_d, w2_d, b2_d, w3_d, b3_d, ts_d, out_d)

    _split_multi_waits(nc)
    return nc


def _emit_body(nc, tc, const, w3pool, b3pool, fsb, cpool, xpool, opool,
               fpsum, mpsum, dram, zt_d, w1_d, b1_d, w2_d, b2_d, w3_d, b3_d,
               ts_d, out_d):
    # ---- constant loads (all host-prearranged partition-major).
    # z/W1 (both fp16, W1 pre-scaled x8) are chunked so the h1 matmuls start
    # as soon as the first quarter lands.
    zchunks, w1chunks = [], []
    for c4 in range(4):
        zc = const.tile([128, 8, B], F16, name=f"zsb{c4}", tag=f"zsb{c4}")
        nc.sync.dma_start(zc[:], zt_d[:, ts(c4, 8), :])
        w1c_t = const.tile([128, 8, H1], F16, name=f"w1sb{c4}", tag=f"w1sb{c4}")
        nc.sync.dma_start(w1c_t[:], w1_d[:, ts(c4, 8), :])
        zchunks.append(zc)
        w1chunks.append(w1c_t)
    w2sb = const.tile([128, H1 // 128, H2], FP, name="w2sb", tag="w2sb")
    nc.sync.dma_start(w2sb[:], w2_d[:, :, :])
    b1sb = const.tile([128, H1 // 128], FP, name="b1sb", tag="b1sb")
    nc.sync.dma_start(b1sb[:], b1_d[:, :])
    b2sb = const.tile([128, H2 // 128], FP, name="b2sb", tag="b2sb")
    nc.sync.dma_start(b2sb[:], b2_d[:, :])
    tssb = const.tile([1, BPC, NPTS], F16, name="tssb", tag="tssb")
    nc.sync.dma_start(tssb[:], ts_d[:, :].rearrange("(a j) n -> a j n", a=1))
    ones16 = const.tile([1, 128], F16, name="ones16", tag="ones16")
    nc.gpsimd.memset(ones16[:], 1.0)

    # ---- h1 = relu(W1 @ z.T + b1), stored [128, 4, 32] fp32.
    # W1/z are fp16 (W1 x8 pre-scaled); the ACT rescales by 1/8 exactly.
    h1sb = const.tile([128, H1 // 128, B], FP, name="h1sb", tag="h1sb")
    for m in range(H1 // 128):
        h1ps = mpsum.tile([128, B], FP, name="h1ps", tag="ps")
        for kt in range(LAT // 128):
            nc.tensor.matmul(
                h1ps[:],
                w1chunks[kt // 8][:, kt % 8, ts(m, 128)],
                zchunks[kt // 8][:, kt % 8, :],
                start=(kt == 0), stop=(kt == LAT // 128 - 1),
            )
        nc.scalar.activation(
            h1sb[:, m, :], h1ps[:], AF.Relu, bias=b1sb[:, m : m + 1],
            scale=1.0 / 8.0,
        )

    # ---- h2h = fp16(32*relu(W2 @ h1 + b2)) as [128, 8, 32]
    h2h = const.tile([128, H2 // 128, B], F16, name="h2h", tag="h2h")
    for m in range(H2 // 128):
        h2ps = mpsum.tile([128, B], FP, name="h2ps", tag="ps")
        for kt in range(H1 // 128):
            nc.tensor.matmul(
                h2ps[:], w2sb[:, kt, ts(m, 128)], h1sb[:, kt, :],
                start=(kt == 0), stop=(kt == H1 // 128 - 1),
            )
        # 32*relu(x + b2) == relu(32x + 32*b2); b2s is pre-scaled on host
        nc.scalar.activation(
            h2h[:, m, :], h2ps[:], AF.Relu, bias=b2sb[:, m : m + 1], scale=SCALE
        )

    # ---- CoordinateNet param tiles (fp16, filled by extraction DMAs)
    win4 = cpool.tile([1, BPC, HID], F16, name="win4", tag="win4")
    bin4 = cpool.tile([128, BPC, 2], F16, name="bin4", tag="bin4")
    wh4s = [
        cpool.tile([128, BPC, 2, HID], F16, name=f"wh4_{l}", tag=f"wh4_{l}")
        for l in range(3)
    ]
    bh4s = [
        cpool.tile([128, BPC, 2], F16, name=f"bh4_{l}", tag=f"bh4_{l}")
        for l in range(3)
    ]
    wo4 = cpool.tile([128, BPC, 2, 4], F16, name="wo4", tag="wo4")
    bo4 = cpool.tile([1, BPC, 4], F16, name="bo4", tag="bo4")

    xs = [None] * BPC

    def _input_layer():
        for j in range(BPC):
            xc = xpool.tile([128, 2, NPTS], F16, name="xt", tag="xt")
            for t in range(2):
                xps = mpsum.tile([128, NPTS], FP, name="xps", tag="ps")
                nc.tensor.matmul(
                    xps[:], win4[0:1, j, ts(t, 128)], tssb[0:1, j, :],
                    start=True, stop=True,
                )
                nc.scalar.activation(
                    xc[:, t, :], xps[:], AF.Relu, bias=bin4[:, j, t : t + 1]
                )
            xs[j] = xc

    def _hidden_layer(l):
        for j in range(BPC):
            xn = xpool.tile([128, 2, NPTS], F16, name="xt", tag="xt")
            for m in range(2):
                hps = mpsum.tile([128, NPTS], FP, name="hps", tag="ps")
                for t in range(2):
                    nc.tensor.matmul(
                        hps[:], wh4s[l][:, j, t, ts(m, 128)], xs[j][:, t, :],
                        start=(t == 0), stop=(t == 1),
                    )
                nc.scalar.activation(
                    xn[:, m, :], hps[:], AF.Relu, bias=bh4s[l][:, j, m : m + 1]
                )
            xs[j] = xn

    def _output_layer():
        # one [128, 16] psum per 128-point chunk m holds all 4 samples' 4 outs
        for m in range(4):
            ops_ = mpsum.tile([128, 4 * BPC], FP, name="ops", tag="ps")
            for j in range(BPC):
                for t in range(2):
                    nc.tensor.matmul(
                        ops_[:, ts(j, 4)], xs[j][:, t, ts(m, 128)],
                        wo4[:, j, t, :], start=(t == 0), stop=False,
                    )
                nc.tensor.matmul(
                    ops_[:, ts(j, 4)], ones16[:, :128], bo4[0:1, j, :],
                    start=False, stop=True,
                )
            outm = opool.tile([128, BPC, 4], FP, name="outm", tag="outm")
            nc.scalar.activation(
                outm[:], ops_[:].rearrange("p (j o) -> p j o", o=4), AF.Sigmoid
            )
            nc.sync.dma_start(
                out_d[:, ts(m, 128), :].rearrange("j p o -> p j o"), outm[:]
            )

    def _extract_small(a2a_out):
        # peer layout (512 cols each): 0: Win|bin, 1: bh0|bh1, 2: bh2|bo|pad,
        # 3: Wo[i<128], 4: Wo[i>=128], 5-7: pad
        v = nc.gpsimd
        v.dma_start(
            win4[:], a2a_out[0:BPC, 0:HID].rearrange("(a r) o -> a r o", a=1)
        )
        for dst, r0, col0 in (
            (bin4, 0, HID),
            (bh4s[0], BPC, 0),
            (bh4s[1], BPC, HID),
            (bh4s[2], 2 * BPC, 0),
        ):
            for t in range(2):
                v.dma_start(
                    dst[:, :, t : t + 1],
                    a2a_out[
                        r0 : r0 + BPC, col0 + 128 * t : col0 + 128 * (t + 1)
                    ].rearrange("r p -> p r"),
                )
        v.dma_start(
            bo4[:],
            a2a_out[2 * BPC : 3 * BPC, HID : HID + 4].rearrange(
                "(a r) o -> a r o", a=1
            ),
        )
        for t in range(2):
            pr = (3 + t) * BPC
            v.dma_start(
                wo4[:, :, t, :],
                a2a_out[pr : pr + BPC, :].rearrange("r (i o) -> i r o", o=4),
            )

    def _extract_wh(l, a2a_out):
        # peer s carries input-rows i in [32s, 32s+32), each row 256 cols;
        # i = 128*t + 32*s4 + il -> partition 32*s4+il, half t.
        # Queue choice: each extraction fires right after its AllToAll, so it
        # must sit on a queue idle at that moment. wh0 fires mid-stream ->
        # Act queue (its flushes are far away); wh1/wh2 fire after the W3
        # stream is done -> SP + Act split (Pool stays free for the next
        # group's staging + collective).
        engs3 = [nc.sync, nc.scalar, nc.gpsimd]
        for t in range(2):
            for s4 in range(4):
                if l < 2:
                    eng = nc.gpsimd
                else:
                    eng = engs3[(4 * t + s4) % 3]
                pr = (4 * t + s4) * BPC
                eng.dma_start(
                    wh4s[l][32 * s4 : 32 * s4 + 32, :, t, :],
                    a2a_out[pr : pr + BPC, :].rearrange(
                        "r (il o) -> il r o", o=HID
                    ),
                )

    # ---- streamed W3 shard matmul + pipelined AllToAll param exchange.
    # PE emission order interleaves CoordinateNet layers where their params
    # are already exchanged, so the in-order PE never stalls.
    a2a_outs = []
    for g in range(4):
        gw = GRP_W[g]
        c0 = GRP_OFF[g]
        nch = NCH[g]
        ntile = (nch + 3) // 4
        fpt = [
            fpsum.tile([128, 512], FP, name=f"fpt{g}_{i}", tag="fps")
            for i in range(ntile)
        ]
        nq = max(1, gw // 2048)       # quarter-stripes: fine-grained DMA FIFO
        qw = gw // nq
        for kk in range(4):
            w3q = []
            for q4 in range(nq):
                w3t = w3pool.tile([128, 2, qw], F16, name="w3sb", tag="w3sb")
                nc.sync.dma_start(
                    w3t[:],
                    w3_d[:, kk, :, c0 + q4 * qw : c0 + (q4 + 1) * qw],
                )
                w3q.append(w3t)
            for t in range(2):
                k = kk * 2 + t
                for j in range(nch):
                    po = 32 * (j % 4)
                    nc.tensor.matmul(
                        fpt[j // 4][po : po + 32, :],
                        h2h[:, k, :],
                        w3q[j // 4][:, t, ts(j % 4, 512)] if nch >= 4
                        else w3q[0][:, t, ts(j, 512)],
                        start=(k == 0),
                        stop=False,
                        tile_position=(0, po),
                    )
            # stage net layers where the PE arrives well after the layer's
            # AllToAll + extraction have completed. tile_wait_until pins the
            # scheduler's placement (its internal collective-cost model is
            # optimistic and would otherwise hoist these into the stream,
            # stalling the in-order PE on the exchange).
            if "nocoord" not in _ABLATE:
                if kk == 1 and g == 2:
                    with tc.tile_wait_until(_VT[0]):
                        _input_layer()
                if kk == 1 and g == 3:
                    with tc.tile_wait_until(_VT[1]):
                        _hidden_layer(0)

        b3sb = b3pool.tile([1, gw], F16, name="b3sb", tag="b3sb")
        nc.sync.dma_start(b3sb[:], b3_d[:, c0 : c0 + gw])
        flat_sb = fsb.tile([128, 512 * ntile], F16, name="flat_sb", tag="flat_sb")
        for j in range(nch):
            po = 32 * (j % 4)
            nc.tensor.matmul(
                fpt[j // 4][po : po + 32, :],
                ones16[:, :B],
                b3sb[:, ts(j, 512)],
                start=False,
                stop=True,
                tile_position=(0, po),
            )
        for t4 in range(ntile):
            # undo the 32*32 pre-scale (exact power of two), cast to fp16;
            # one ACT flushes a whole PSUM tile (all 4 sample-slots)
            nc.scalar.mul(
                flat_sb[:, ts(t4, 512)] if nch >= 4 else flat_sb[0:32, 0:512],
                fpt[t4][:] if nch >= 4 else fpt[t4][0:32, :],
                1.0 / 1024.0,
            )
        a2a_in = dram.tile([B, gw], F16, name=f"a2ain{g}", tag=f"a2ain{g}")
        a2a_out = dram.tile([B, gw], F16, name=f"a2aout{g}", tag=f"a2aout{g}")
        fw = 512 * ntile
        for f in range(min(4, nch)):
            # the last group's staging is latency-critical (it gates the
            # final AllToAll after the stream is over): use the then-idle
            # SP/Act HWDGE queues there instead of serial Pool SWDGE gens
            seng = (nc.sync if f % 2 == 0 else nc.scalar) if g == 3 else nc.gpsimd
            seng.dma_start(
                a2a_in[:, f * fw : (f + 1) * fw] if nch >= 4
                else a2a_in[:, 0:gw],
                flat_sb[32 * f : 32 * f + 32, 0:fw] if nch >= 4
                else flat_sb[0:32, 0:gw],
            )
        if "nocoll" in _ABLATE:
            a2a_out = a2a_in
        else:
            nc.gpsimd.collective_compute(
                "AllToAll",
                mybir.AluOpType.bypass,
                replica_groups=[list(range(NCORES))],
                ins=[a2a_in.opt()],
                outs=[a2a_out.opt()],
            )
        a2a_outs.append(a2a_out)
        if "nocoord" not in _ABLATE:
            if g == 0:
                _extract_small(a2a_out)
            elif g == 1:
                _extract_wh(0, a2a_out)
            elif g == 3:
                # wh1's extraction is emitted here -- after group 3's staging
                # and AllToAll are queued -- so its descriptor generation
                # cannot delay them on the Pool queue; hidden layer 1 then
                # runs on the PE while the last AllToAll is in flight.
                _extract_wh(1, a2a_outs[2])
                with tc.tile_wait_until(_VT[2]):
                    _hidden_layer(1)
                _extract_wh(2, a2a_out)

    if "nocoord" not in _ABLATE:
        with tc.tile_wait_until(_VT[3]):
            _hidden_layer(2)
        with tc.tile_wait_until(_VT[4]):
            _output_layer()


_NC_CACHE = {}


def _get_module(repeat: int = 1):
    if repeat not in _NC_CACHE:
        _NC_CACHE[repeat] = _build_module(repeat)
    return _NC_CACHE[repeat]


# -------------------------------------------------------------- host wrapper
def _core_gcols(c):
    """Global W3-column index (or -1 for zero pad) of every streamed column,
    in per-core storage order."""
    small = np.full((NCORES, SW), -1, np.int64)
    small[0] = np.arange(512)                       # Win (o-major) | bin
    small[1, 0:HID] = BH_OFF[0] + np.arange(HID)    # bh0
    small[1, HID:2 * HID] = BH_OFF[1] + np.arange(HID)  # bh1
    small[2, 0:HID] = BH_OFF[2] + np.arange(HID)    # bh2
    small[2, HID : HID + 4] = BO_OFF + np.arange(4)  # bo
    for pk in (3, 4):
        ii = (pk - 3) * 128 + np.arange(128)
        # Wo i-major: col (i, o) <- WO_OFF + o*HID + i
        small[pk] = (WO_OFF + np.arange(4)[None, :] * HID + ii[:, None]).ravel()

    # Wh blocks: storage chunk j holds param chunk k = (j%4)*4 + j//4 so the
    # PSUM partition-slot staging lands the A2A buffer in plain param order.
    j = np.arange(16)
    k = (j % 4) * 4 + j // 4
    s_of_store = (k[:, None] * 512 + np.arange(512)[None, :]).ravel()
    il, o = s_of_store // HID, s_of_store % HID
    parts = [small[c]]
    for l in range(3):
        parts.append(WH_OFF[l] + o * HID + (32 * c + il))
    return np.concatenate(parts)


_PREP_CACHE = None
LAST_RESULTS = None


def prepare_in_maps(z, timestamps, W1, b1, W2, b2, W3, b3):
    z = np.asarray(z, np.float32)
    timestamps = np.asarray(timestamps, np.float32)
    W1 = np.asarray(W1, np.float32)
    b1 = np.asarray(b1, np.float32)
    W2 = np.asarray(W2, np.float32)
    b2 = np.asarray(b2, np.float32)
    W3 = np.asarray(W3, np.float32)
    b3 = np.asarray(b3, np.float32)

    zt_r = np.ascontiguousarray(
        z.T.reshape(LAT // 128, 128, B).transpose(1, 0, 2)
    ).astype(np.float16)
    w1r = np.ascontiguousarray(
        (8.0 * W1).T.reshape(LAT // 128, 128, H1).transpose(1, 0, 2)
    ).astype(np.float16)
    b1f = np.ascontiguousarray(b1.reshape(H1 // 128, 128).T)
    w2r = np.ascontiguousarray(
        W2.T.reshape(H1 // 128, 128, H2).transpose(1, 0, 2)
    )
    b2s = np.ascontiguousarray((SCALE * b2).reshape(H2 // 128, 128).T)

    in_maps = []
    for c in range(NCORES):
        gcols = _core_gcols(c)
        valid = gcols >= 0
        gc = np.where(valid, gcols, 0)
        wsel = SCALE * W3[gc, :]
        wsel[~valid] = 0.0
        w3p = np.ascontiguousarray(
            wsel.T.reshape(4, 2, 128, SCOLS).transpose(2, 0, 1, 3)
        ).astype(np.float16)
        b3sel = 1024.0 * b3[gc]
        b3sel[~valid] = 0.0
        b3r = b3sel.astype(np.float16).reshape(1, SCOLS)
        in_maps.append(
            {
                "zt_r": zt_r,
                "w1r": w1r,
                "b1f": b1f,
                "w2r": w2r,
                "b2s": b2s,
                "w3p": w3p,
                "b3r": b3r,
                "tst": np.ascontiguousarray(
                    timestamps[c * BPC : (c + 1) * BPC, :, 0]
                ).astype(np.float16),
            }
        )
    return in_maps


def kernel(z, timestamps, W1, b1, W2, b2, W3, b3):
    global LAST_RESULTS
    in_maps = prepare_in_maps(z, timestamps, W1, b1, W2, b2, W3, b3)
    nc = _get_module(1)
    res = run_bass_kernel_spmd(nc, in_maps, core_ids=list(range(NCORES)))
    LAST_RESULTS = res
    out = np.concatenate(
        [np.asarray(res.results[c]["out"]) for c in range(NCORES)], axis=0
    )
    return out.astype(np.float32, copy=False)
